# revision 42
# baseline (speedup 1.0000x reference)
"""Multi-head attention (B=4, S=2048, D=1024, H=16, HD=64) on 8 TRN2 NeuronCores.

Sharding: core c handles batch b = c//2 and head-group hg = c%2 (8 heads each).
Attention is embarrassingly parallel over (b, head-group); the QKV projection is
column-sharded per core (tensor parallel on heads).

Per-core dataflow (transposed layout, fp16 operands, fp32 accumulation):
  - Host passes X^T [D, S], W slices (fp16) in natural [D, cols] layout.
  - Projection:  Q^T/K^T [1024, S] = W_qk^T @ X accumulated into SBUF m-tiles
                 (f32r) via PE->PSUM->DVE; V [S, 512] = X @ W_v kept in SBUF
                 (fp16) augmented with a ones-column per head (V').
  - Per head:    S^T[k,q] = K^T.T @ Q^T (PSUM f32), st = exp(S^T/8) (ScalarE,
                 fp16 out; mask all-ones => shift-invariant, no max pass),
                 out^T[d,q], sums[q] = V'^T @ st (PSUM accumulation over the 16
                 seq chunks; the ones-row yields softmax sums), then
                 out^T /= sums (DVE reciprocal + gpsimd partition broadcast +
                 DVE multiply) and fp16 DMA out per pass.

fp16 (not bf16): same 1 row/cycle matmul speed and half-size DMA, but 10
mantissa bits keep the end-to-end error at ~3e-3 mean relative (bf16's 7
bits measured 2.3e-2, over the 2e-2 gate).

Schedule: the ScalarE exp train is the kernel's clock, and the score-PSUM
ring depth is the only elasticity between PE and ScalarE — so scores are
computed in [128,512] PSUM tiles (ring of 4, same 4 banks as 2x1024) with
one [128,512] exp per QK matmul: ScalarE pays +25us of per-instruction
overhead but the 2-unit lookahead window (~740ns) absorbs the woven
projection quanta that a 2-deep ring turned 1:1 into train stretch.

Heads run as 4 pairs; within a pair the two heads' (pass, chunk) units
interleave, and each head does two q-half passes (qh-major) so the normalize
of pass p hides under pass p+1 and the four PSUM av banks rotate 2+2 across
the interleaved heads. Projection is cut into 0.2-0.9us quanta threaded
through the chunk stream by a coupled-frontier scheduler (QK gated by
exp(u-2), exp by QK(u) and exp(u-1); emit a quantum when the PE frontier
would stall, subject to readiness = DMA landing + sbt ring aliasing, and
due-dates = first consuming unit, staggered so forced quanta never pile on
one unit): V lands just-in-time in the pair of its first consuming head
(split by head-pair), each pair's Q-side m-tile right before the pair and
its K-side m-tile *inside* the pair (K columns for chunk kc are only read
at chunk kc). DMAs are few large copies ordered so the first QK fires ~11us
in; per-pair W m-tiles stream lazily. Pass/pair boundaries early-flush the
even head and defer its normalize multiplies behind the odd head's
reciprocals so the DVE tail chains interleave.

All matmuls fp16 or f32r (1 row/cycle). b_qkv is applied (zeros in
practice); mask is all-True per the problem spec and is ignored.
"""

import numpy as np
import ml_dtypes

import concourse.bass as bass
import concourse.mybir as mybir
import concourse.tile as tile
from concourse import bacc
from concourse.bass_utils import run_bass_kernel_spmd

F32 = mybir.dt.float32
F32R = mybir.dt.float32r
BF16 = mybir.dt.bfloat16
F16 = mybir.dt.float16
AF = mybir.ActivationFunctionType
ALU = mybir.AluOpType

P = 128          # partitions
D = 1024         # model dim
S = 2048         # sequence
HD = 64          # head dim
NHC = 8          # heads per core
QKC = NHC * HD   # 512 columns per core for each of Q, K, V
KD = D // P      # 8 contraction chunks
MS = S // P      # 16 sequence chunks
SCALE = 1.0 / 8.0  # 1/sqrt(HD)

N_CORES = 8
B_FULL, H_FULL = 4, 16

BF = ml_dtypes.bfloat16
NPF16 = np.float16

# cost-model constants for the weave load balancer (ns)
ACT_UNIT = 1223.0       # two [128,512] exps (512-wide keeps the sc ring 4 deep)
PE_UNIT = 852.0         # QK (2x512 cols) + AV (2x512 cols)
MM_COL = 512 * 0.4167   # one 512-col matmul


def _build(iters=1):
    nc = bacc.Bacc(None, target_bir_lowering=False)

    xt = nc.dram_tensor("xt", [D, S], F16, kind="ExternalInput")
    # wqk is host-permuted: row (m*128 + p), col (k*128 + j) holds
    # W_qk[k*128 + p, m*128 + j] — one m-tile's weights are a contiguous
    # [128, 1024] block
    wqk = nc.dram_tensor("wqk", [D, 2 * QKC], F16, kind="ExternalInput")
    wv = nc.dram_tensor("wv", [D, QKC], F16, kind="ExternalInput")
    bqk = nc.dram_tensor("bqk", [2 * QKC], F32, kind="ExternalInput")
    bv = nc.dram_tensor("bv", [QKC], F32, kind="ExternalInput")
    outT = nc.dram_tensor("outT", [QKC, S], F16, kind="ExternalOutput")

    xt4 = xt[:].rearrange("(k p) s -> p k s", p=P)
    wv4 = wv[:].rearrange("(k p) n -> p k n", p=P)

    with tile.TileContext(nc) as tc:
        with (
            tc.tile_pool(name="persist", bufs=1) as pp,
            tc.tile_pool(name="sbtp", bufs=4) as sbtp,
            tc.tile_pool(name="stp", bufs=8) as stp,
            tc.tile_pool(name="denp", bufs=4) as denp,
            tc.tile_pool(name="bcp", bufs=4) as bcp,
            tc.tile_pool(name="psc", bufs=4, space="PSUM") as psc,
            tc.tile_pool(name="psav", bufs=4, space="PSUM") as psav,
        ):
            # Warm the Exp activation table at t=0 so the table load is off
            # the critical path of the first real exp.
            warm = pp.tile([1, 8], F32, tag="warm", name="warm")
            nc.vector.memset(warm[:], 0.0)
            nc.scalar.activation(warm[:], warm[:], AF.Exp, scale=1.0)
            ones8 = pp.tile([P, NHC], F16, tag="ones8", name="ones8")
            nc.vector.memset(ones8[:], 1.0)

            bqk_sb = pp.tile([P, KD], F32, tag="bqk", name="bqk_sb")
            bv_bc = pp.tile([P, QKC], F32, tag="bvb", name="bv_bc")

            for it in range(iters):
                # V' tiles: [128 seq, 8 heads, 64+1] with ones in the last col
                v_sb = [
                    pp.tile([P, NHC, HD + 1], F16, tag=f"v{k}", name=f"v{it}_{k}")
                    for k in range(MS)
                ]

                with tc.tile_pool(name=f"proj{it}", bufs=1) as pj:
                    w_tiles = {}

                    def load_wm(m, it=it):
                        w_tiles[m] = pj.tile([P, KD, P], F16, tag="wm", bufs=4,
                                             name=f"wm{it}_{m}")
                        nc.sync.dma_start(
                            out=w_tiles[m][:],
                            in_=wqk[m * P:(m + 1) * P, :].rearrange("p (k j) -> p k j", k=KD))

                    xt_sb = pj.tile([P, KD, S], F16, tag="xt", name=f"xt{it}")
                    wv_sb = pj.tile([P, KD, QKC], F16, tag="wv", name=f"wv{it}")

                    # DMA order (each copy ~650ns to issue, transfers serialize
                    # on the DMA engines): wm0, wm4, biases, xt cols 0-1023 in
                    # 4 k-pair copies (gates the prefix), wv cols 0-255 (V for
                    # heads 0-3), xt cols 1024-2047, wv cols 256-511.
                    load_wm(0)
                    load_wm(4)
                    for kp in range(4):
                        nc.sync.dma_start(out=xt_sb[:, 2 * kp:2 * kp + 2, 0:S // 2],
                                          in_=xt4[:, 2 * kp:2 * kp + 2, 0:S // 2])
                    nc.sync.dma_start(out=bqk_sb[:],
                                      in_=bqk[:].rearrange("(m p) -> p m", p=P))
                    nc.sync.dma_start(out=bv_bc[0:1, :],
                                      in_=bv[:].rearrange("(o n) -> o n", o=1))
                    nc.gpsimd.partition_broadcast(bv_bc[:], bv_bc[0:1, :])
                    nc.sync.dma_start(out=wv_sb[:, :, 0:QKC // 2],
                                      in_=wv4[:, :, 0:QKC // 2])
                    nc.sync.dma_start(out=xt_sb[:, :, S // 2:S],
                                      in_=xt4[:, :, S // 2:S])
                    nc.sync.dma_start(out=wv_sb[:, :, QKC // 2:QKC],
                                      in_=wv4[:, :, QKC // 2:QKC])

                    sbt_tiles = {}

                    def get_sbt(m, it=it):
                        if m not in sbt_tiles:
                            sbt_tiles[m] = sbtp.tile([P, S], F32R, tag="sbt",
                                                     name=f"sbt{it}_{m}")
                        return sbt_tiles[m]

                    def quantum_mm(m, quarter, k0, nk, first, it=it):
                        """nk contraction chunks of one 512-col quarter of Q/K
                        m-tile m, accumulated into sbt via PE->PSUM->DVE."""
                        if m not in w_tiles:
                            load_wm(m)
                        w_m, sbt = w_tiles[m], get_sbt(m)
                        ps = psc.tile([P, 512], F32, tag="sc",
                                      name=f"pq{it}_{m}_{quarter}_{k0}")
                        for j, k in enumerate(range(k0, k0 + nk)):
                            nc.tensor.matmul(
                                ps[:], w_m[:, k, :],
                                xt_sb[:, k, quarter * 512:(quarter + 1) * 512],
                                start=(j == 0), stop=(j == nk - 1))
                        dst = sbt[:, quarter * 512:(quarter + 1) * 512]
                        if first:
                            nc.vector.tensor_scalar_add(dst, ps[:], bqk_sb[:, m:m + 1])
                        else:
                            nc.vector.tensor_tensor(out=dst, in0=ps[:], in1=dst, op=ALU.add)

                    def quantum_mm_cols(m, c0, w, k0, nk, first, it=it):
                        """nk contraction chunks of cols [c0, c0+w) of m-tile m."""
                        if m not in w_tiles:
                            load_wm(m)
                        w_m, sbt = w_tiles[m], get_sbt(m)
                        ps = psc.tile([P, w], F32, tag="sc",
                                      name=f"pqc{it}_{m}_{c0}_{k0}")
                        for j, k in enumerate(range(k0, k0 + nk)):
                            nc.tensor.matmul(ps[:], w_m[:, k, :],
                                             xt_sb[:, k, c0:c0 + w],
                                             start=(j == 0), stop=(j == nk - 1))
                        dst = sbt[:, c0:c0 + w]
                        if first:
                            nc.vector.tensor_scalar_add(dst, ps[:], bqk_sb[:, m:m + 1])
                        else:
                            nc.vector.tensor_tensor(out=dst, in0=ps[:], in1=dst, op=ALU.add)

                    def v_quantum(ms, vh, k0, nk, first, nh=4, it=it, v_sb=v_sb):
                        """nk contraction chunks of the V projection for seq
                        chunk ms, heads [4vh, 4vh+nh) (nh=2 gives a head-pair
                        quarter quantum)."""
                        ps = psc.tile([P, 64 * nh], F32, tag="sc",
                                      name=f"pv{it}_{ms}_{vh}_{k0}")
                        c0 = vh * 256
                        for j, k in enumerate(range(k0, k0 + nk)):
                            nc.tensor.matmul(
                                ps[:], xt_sb[:, k, ms * P:(ms + 1) * P],
                                wv_sb[:, k, c0:c0 + 64 * nh],
                                start=(j == 0), stop=(j == nk - 1))
                        h0 = 4 * vh
                        dst = v_sb[ms][:, h0:h0 + nh, 0:HD]
                        src3 = ps[:].rearrange("p (h e) -> p h e", e=HD)
                        if first:
                            nc.vector.tensor_tensor(
                                out=dst, in0=src3,
                                in1=bv_bc[:, c0:c0 + 64 * nh].rearrange(
                                    "p (h e) -> p h e", e=HD),
                                op=ALU.add)
                            nc.vector.tensor_copy(
                                v_sb[ms][:, h0:h0 + nh, HD:HD + 1],
                                ones8[:, h0:h0 + nh].rearrange("p (h o) -> p h o", o=1))
                        else:
                            nc.vector.tensor_tensor(out=dst, in0=src3, in1=dst, op=ALU.add)

                    def v_quantum_hp(ms, hp, k0, nk, first, it=it, v_sb=v_sb):
                        """One head-pair slice (heads 2hp, 2hp+1) of V chunk ms."""
                        c0 = 128 * hp
                        ps = psc.tile([P, 128], F32, tag="sc", name=f"pw{it}_{ms}_{hp}_{k0}")
                        for j, k in enumerate(range(k0, k0 + nk)):
                            nc.tensor.matmul(
                                ps[:], xt_sb[:, k, ms * P:(ms + 1) * P],
                                wv_sb[:, k, c0:c0 + 128],
                                start=(j == 0), stop=(j == nk - 1))
                        dst = v_sb[ms][:, 2 * hp:2 * hp + 2, 0:HD]
                        src3 = ps[:].rearrange("p (h e) -> p h e", e=HD)
                        if first:
                            nc.vector.tensor_tensor(
                                out=dst, in0=src3,
                                in1=bv_bc[:, c0:c0 + 128].rearrange("p (h e) -> p h e", e=HD),
                                op=ALU.add)
                            nc.vector.tensor_copy(
                                v_sb[ms][:, 2 * hp:2 * hp + 2, HD:HD + 1],
                                ones8[:, 2 * hp:2 * hp + 2].rearrange("p (h o) -> p h o", o=1))
                        else:
                            nc.vector.tensor_tensor(out=dst, in0=src3, in1=dst, op=ALU.add)

                    # ---- prefix: m0 q0/q1 + m4 q0 trickled k-major behind
                    # the xt DMA stream (3 concurrent full-depth PSUM groups
                    # in the av ring; attention has not claimed it yet)
                    g_pf = [psav.tile([P, 512], F32, tag="av", name=f"pf{it}_{i}")
                            for i in range(2)]
                    g_m4 = psav.tile([P, 256], F32, tag="av", name=f"pf{it}_m4")
                    g_m4b = psav.tile([P, 256], F32, tag="av", name=f"pf{it}_m4b")

                    def pf_blk(k):
                        nc.tensor.matmul(g_pf[0][:], w_tiles[0][:, k, :],
                                         xt_sb[:, k, 0:512],
                                         start=(k == 0), stop=(k == KD - 1))
                        nc.tensor.matmul(g_pf[1][:], w_tiles[0][:, k, :],
                                         xt_sb[:, k, 512:1024],
                                         start=(k == 0), stop=(k == KD - 1))
                        nc.tensor.matmul(g_m4[:], w_tiles[4][:, k, :],
                                         xt_sb[:, k, 0:256],
                                         start=(k == 0), stop=(k == KD - 1))
                        nc.tensor.matmul(g_m4b[:], w_tiles[4][:, k, :],
                                         xt_sb[:, k, 256:512],
                                         start=(k == 0), stop=(k == KD - 1))

                    for k in range(KD):
                        pf_blk(k)
                    # finalize order: the first QK matmul reads kt (m4, K
                    # chunks 0-1 suffice) and qt cols 0-511 (m0 q0); the
                    # second half adds m0 q1.
                    nc.vector.tensor_scalar_add(get_sbt(4)[:, 0:256], g_m4[:],
                                                bqk_sb[:, 4:5])
                    nc.vector.tensor_scalar_add(get_sbt(0)[:, 0:512], g_pf[0][:],
                                                bqk_sb[:, 0:1])
                    nc.vector.tensor_scalar_add(get_sbt(0)[:, 512:1024], g_pf[1][:],
                                                bqk_sb[:, 0:1])
                    nc.vector.tensor_scalar_add(get_sbt(4)[:, 256:512], g_m4b[:],
                                                bqk_sb[:, 4:5])
                    # V seq chunks 0-1, heads 0-1: consumed by AV(h0, kc=0/1);
                    # their matmuls overlap the DVE finalizes above.
                    v_quantum(0, 0, 0, KD, first=True, nh=2)
                    v_quantum(1, 0, 0, KD, first=True, nh=2)


                    # ---- weave quanta: (due_unit, order, ready_unit, cost, fn).
                    # All quanta are FULL-depth (one PSUM group, one DVE
                    # finalize): short-lived groups round-trip PE->DVE->PE
                    # inside the 2-slot sc ring and serialize the QK stream,
                    # while a full 8-matmul group self-spaces past the DVE
                    # latency. due = a couple units before the first consuming
                    # unit; ready = DMA landing / sbt-ring-aliasing safe point.
                    quanta = []

                    def add_q(due, ready, cost, fn):
                        quanta.append((due, len(quanta), ready, cost, fn))

                    # V heads 0-3: all 16 seq chunks inside pair 0 (AV(h0,ms)
                    # executes at unit 2ms+2); V heads 4-7 due pair 2 but
                    # allowed to smear back through pairs 1-2.
                    # V heads 0-1 JIT inside pair 0 (AV(h0, ms) at unit
                    # 2ms+2); heads 2-3 defer to pair 1 where first consumed
                    # (fp16 matmuls run 1 row/cycle at any width)
                    for ms in range(2, MS):
                        add_q(max(2 * ms - 3, 0), 0 if ms < 8 else 2, MM_COL,
                              lambda ms=ms: v_quantum(ms, 0, 0, KD, first=True, nh=2))
                    for ms in range(1, MS):
                        add_q(64 + 2 * ms - 3, max(2, 10 + 2 * ms), MM_COL,
                              lambda ms=ms: v_quantum_hp(ms, 1, 0, KD, first=True))
                    for ms in range(MS):
                        add_q(max(126 + 2 * ms - 3, 126), max(6, 26 + 2 * ms), MM_COL,
                              lambda ms=ms: v_quantum(ms, 1, 0, KD, first=True, nh=2))
                    for ms in range(MS):
                        add_q(max(190 + 2 * ms - 3, 190), 76 + 2 * ms, MM_COL,
                              lambda ms=ms: v_quantum_hp(ms, 3, 0, KD, first=True))
                    # pair 0 K-side remainder (m4 q1-3, due at chunk 4j) and
                    # Q-side pass-1 quarters (m0 q2-3, due at slot 32); halves
                    # staggered so forced dues never pile onto one unit.
                    for j in (1, 2, 3):
                        for k0 in (0, 4):
                            add_q(8 * j - 6 + k0 // 2, 0 if j == 1 else 2, 4 * MM_COL,
                                  lambda j=j, k0=k0: quantum_mm(4, j, k0, 4,
                                                                first=(k0 == 0)))
                    add_q(61, 40, MM_COL,
                          lambda: v_quantum_hp(0, 1, 0, KD, first=True))
                    for j in (2, 3):
                        for k0 in (0, 4):
                            add_q(22 + 2 * (j - 2) * 2 + 2 * (k0 // 4), 2, 4 * MM_COL,
                                  lambda j=j, k0=k0: quantum_mm(0, j, k0, 4,
                                                                first=(k0 == 0)))
                    # pairs 1-3: Q-side quarters 0-1 + K-side quarter 0 before
                    # the pair; K-side quarters 1-3 inside the pair (due chunk
                    # 4j); Q-side quarters 2-3 before the pair's pass 1.
                    # ready: sbt tile of pair p aliases pair p-2's (ring of 4),
                    # clamped near the due date so earlier pairs don't strip
                    # later pairs' supply and leave their units bare.
                    for p in (1, 2, 3):
                        base = 64 * p
                        rdy0 = max(64 * (p - 1) + 4, 6)
                        for j in (0, 1):
                            for k0 in (0, 4):
                                add_q(base - 8 + 2 * j, max(rdy0, base - 26),
                                      4 * MM_COL,
                                      lambda p=p, j=j, k0=k0: quantum_mm(
                                          p, j, k0, 4, first=(k0 == 0)))
                        for k0 in (0, 4):
                            add_q(base - 3, max(rdy0, base - 26), 4 * MM_COL,
                                  lambda p=p, k0=k0: quantum_mm(
                                      4 + p, 0, k0, 4, first=(k0 == 0)))
                        for j in (1, 2, 3):
                            for k0 in (0, 4):
                                add_q(base + 8 * j - 6 + k0 // 2,
                                      max(rdy0, base - 10), 4 * MM_COL,
                                      lambda p=p, j=j, k0=k0: quantum_mm(
                                          4 + p, j, k0, 4, first=(k0 == 0)))
                        for j in (2, 3):
                            for k0 in (0, 4):
                                add_q(base + 22 + 2 * (j - 2) * 2 + 2 * (k0 // 4),
                                      max(rdy0, base - 10), 4 * MM_COL,
                                      lambda p=p, j=j, k0=k0: quantum_mm(
                                          p, j, k0, 4, first=(k0 == 0)))

                    quanta.sort(key=lambda q: (q[0], q[1]))
                    pending = list(quanta)
                    # Coupled-frontier model: QK(u) is gated by exp(u-2)
                    # releasing its sc slot; exp(u) by QK(u) and exp(u-1).
                    # Emit a quantum exactly when the PE frontier would
                    # otherwise stall at the next gate, with a cost-based
                    # cooldown so the sc ring turnaround (quantum matmuls +
                    # DVE finalize) stays ahead of the QK allocations.
                    clock = {"pe": 0.0, "cool": -10}
                    exp_done = []
                    SEM = 120.0

                    def unit_clock(u):
                        gate = exp_done[u - 2] + SEM if u >= 2 else 0.0
                        qk_end = max(clock["pe"], gate) + 426
                        prev_exp = exp_done[u - 1] if u >= 1 else 0.0
                        exp_done.append(max(prev_exp, qk_end + SEM) + ACT_UNIT)
                        clock["pe"] = qk_end + 426

                    def weave(u):
                        # forced: everything at/past its due date
                        while pending and pending[0][0] <= u:
                            q = pending.pop(0)
                            q[4]()
                            clock["pe"] += q[3]
                            clock["cool"] = u + 1
                        gap = (exp_done[u - 1] + SEM - clock["pe"]) if u >= 1 else 0
                        if u <= clock["cool"] or gap <= 0:
                            return
                        # voluntary: fill the gap before the next QK's gate
                        for idx, q in enumerate(pending):
                            if q[2] <= u:
                                pending.pop(idx)
                                q[4]()
                                clock["pe"] += q[3]
                                clock["cool"] = u + 1
                                return

                    # ---------------- attention ----------------
                    avs_cur = {}
                    prev = {}

                    def emit_av(h, kc, st, it=it, v_sb=v_sb):
                        avs = avs_cur[h]
                        for i in range(2):
                            nc.tensor.matmul(
                                avs[i][:], v_sb[kc][:, h, :],
                                st[:, i * 512:(i + 1) * 512],
                                start=(kc == 0), stop=(kc == MS - 1))

                    def norm_recip(h, qh, it=it):
                        avs = avs_cur[h]
                        denbs = []
                        for i in range(2):
                            denb = denp.tile([HD, 512], F32, tag="den",
                                             name=f"den{it}_{h}_{qh}_{i}")
                            nc.vector.reciprocal(denb[0:1, :], avs[i][HD:HD + 1, :])
                            denbs.append(denb)
                        for i in range(2):
                            nc.gpsimd.partition_broadcast(denbs[i][:], denbs[i][0:1, :])
                        return denbs

                    def norm_mul(h, qh, denbs, split=False, it=it):
                        avs = avs_cur[h]
                        bc = bcp.tile([HD, 1024], F16, tag="bc",
                                      name=f"bc{it}_{h}_{qh}")
                        for i in range(2):
                            nc.vector.tensor_tensor(out=bc[:, i * 512:(i + 1) * 512],
                                                    in0=avs[i][0:HD, :],
                                                    in1=denbs[i][:], op=ALU.mult)
                            if split:
                                nc.sync.dma_start(
                                    out=outT[h * HD:(h + 1) * HD,
                                             qh * 1024 + i * 512:qh * 1024 + (i + 1) * 512],
                                    in_=bc[:, i * 512:(i + 1) * 512])
                        if not split:
                            nc.sync.dma_start(
                                out=outT[h * HD:(h + 1) * HD, qh * 1024:(qh + 1) * 1024],
                                in_=bc[:])

                    def normalize(h, qh, it=it):
                        norm_mul(h, qh, norm_recip(h, qh))

                    def flush_prev(h, defer=False, it=it):
                        pkc, pst, pqh = prev.pop(h)
                        if pkc == 0:
                            avs_cur[h] = [
                                psav.tile([HD + 1, 512], F32, tag="av",
                                          name=f"av{it}_{h}_{pqh}_{i}")
                                for i in range(2)
                            ]
                        emit_av(h, pkc, pst)
                        if pkc == MS - 1:
                            if defer:
                                return pqh, norm_recip(h, pqh)
                            normalize(h, pqh)
                        return None

                    for g in range(4):
                        for s in range(64):
                            h = 2 * g + (s & 1)
                            qh = s // 32
                            kc = (s % 32) // 2
                            u = 64 * g + s
                            off = (h % 2) * HD
                            if s == 31 and 2 * g in prev:
                                # even head's pass-0 c15 was QK'd at s=30:
                                # flush now so its normalize frees the av
                                # slots a unit earlier for its pass 1
                                flush_prev(2 * g)
                            if s == 63:
                                # the even head's last chunk was QK'd at s=62;
                                # flush it now (reciprocals only — the muls
                                # queue after the odd head's reciprocals so
                                # the DVE tail chains don't serialize)
                                ev_norm = flush_prev(2 * g, defer=True)
                            qt = get_sbt(g)
                            kt = get_sbt(4 + g)
                            st = stp.tile([P, 1024], F16, tag="st",
                                          name=f"st{it}_{h}_{qh}_{kc}")
                            for half in range(2):
                                sc = psc.tile([P, 512], F32, tag="sc",
                                              name=f"sc{it}_{h}_{qh}_{kc}_{half}")
                                nc.tensor.matmul(
                                    sc[:],
                                    kt[off:off + HD, kc * P:(kc + 1) * P],
                                    qt[off:off + HD,
                                       qh * 1024 + half * 512:qh * 1024 + half * 512 + 512],
                                    start=True, stop=True)
                                nc.scalar.activation(st[:, half * 512:(half + 1) * 512],
                                                     sc[:], AF.Exp, scale=SCALE)
                            unit_clock(u)
                            if h in prev:
                                flush_prev(h)
                            prev[h] = (kc, st, qh)
                            weave(u)
                        # drain the pair's last chunk: odd head's AV +
                        # reciprocals, then both heads' muls + output DMAs
                        od_norm = flush_prev(2 * g + 1, defer=True)
                        if ev_norm is not None:
                            norm_mul(2 * g, ev_norm[0], ev_norm[1], split=True)
                        if od_norm is not None:
                            norm_mul(2 * g + 1, od_norm[0], od_norm[1], split=True)

                    assert not pending, f"unemitted quanta: {len(pending)}"

    nc.finalize()
    return nc


_NC_CACHE = {}


def _get_nc(iters=1):
    if iters not in _NC_CACHE:
        _NC_CACHE[iters] = _build(iters)
    return _NC_CACHE[iters]


def _permute_wqk(wqk):
    # [k*128+p, m*128+j] -> [m*128+p, k*128+j]: one m-tile contiguous per row
    w4 = wqk.reshape(KD, P, KD, P)
    return np.ascontiguousarray(w4.transpose(2, 1, 0, 3).reshape(D, D))


def make_in_maps(inputs, W_qkv, b_qkv):
    inputs = np.asarray(inputs, dtype=np.float32)
    W = np.asarray(W_qkv, dtype=np.float32)
    b = np.asarray(b_qkv, dtype=np.float32)
    xt_by_b = [np.ascontiguousarray(inputs[bi].T).astype(NPF16) for bi in range(B_FULL)]
    in_maps = []
    for c in range(N_CORES):
        bi, hg = c // 2, c % 2
        c0 = hg * QKC
        in_maps.append({
            "xt": xt_by_b[bi],
            "wqk": _permute_wqk(
                np.concatenate([W[:, c0:c0 + QKC], W[:, D + c0: D + c0 + QKC]],
                               axis=1)).astype(NPF16),
            "wv": np.ascontiguousarray(
                W[:, 2 * D + c0: 2 * D + c0 + QKC]).astype(NPF16),
            "bqk": np.ascontiguousarray(
                np.concatenate([b[c0:c0 + QKC], b[D + c0: D + c0 + QKC]])),
            "bv": np.ascontiguousarray(b[2 * D + c0: 2 * D + c0 + QKC]),
        })
    return in_maps


def assemble(results, B=B_FULL):
    out = np.empty((B, S, D), dtype=np.float32)
    for c in range(N_CORES):
        bi, hg = c // 2, c % 2
        out[bi, :, hg * QKC:(hg + 1) * QKC] = \
            np.asarray(results[c]["outT"]).astype(np.float32).T
    return out


def kernel(inputs, mask, W_qkv, b_qkv):
    # mask is all-True for this problem (spec: fill=ones); it does not affect softmax.
    nc = _get_nc()
    in_maps = make_in_maps(inputs, W_qkv, b_qkv)
    res = run_bass_kernel_spmd(nc, in_maps, core_ids=list(range(N_CORES)))
    return assemble(res.results)
